# revision 9
# baseline (speedup 1.0000x reference)
"""Attention-based aggregation (ragged segment-sum) on 8 trn2 NeuronCores.

Strategy: shard the 262144-row instance dim evenly across 8 cores (32768
rows/core). Host pre-tiles features+att into fp16 SBUF-layout arrays
([partition, tile, col] with a ones column appended to features). Each core
streams 256 row-tiles through TensorE: matmul att_tile^T [128,8] @
[feat_tile | 1] [128,129] -> [8,129] fp32 in PSUM (weighted feature sums +
attention sums in one shot). PE column-tiling runs 4 such matmuls
concurrently in the 4 32-column strips of the array, each strip
accumulating Q=8 consecutive tiles -> one PSUM bank holds 4 chunk partials
of 1024 rows at partitions {0-7,32-39,64-71,96-103}. One DVE copy per bank
-> SBUF laid out across all 128 partitions -> single fast DMA out. Host
maps chunk partials onto bags (segment_ids are sorted, so only chunks
straddling a bag boundary are recomputed on host) and does the tiny
[B,8,128] division / NaN fixups.
"""

import numpy as np

TOTAL = 262144
D = 128
H = 8
NCORES = 8
R = TOTAL // NCORES          # rows per core = 32768
TILE = 128                   # rows per matmul tile (contraction dim)
Q = 8                        # tiles accumulated per PSUM chunk
CHROWS = TILE * Q            # 1024 rows per chunk
NSTRIP = 4                   # PE column strips used concurrently
CG = NSTRIP * Q              # tiles per chunk-group (= per PSUM bank) = 32
DP = D + 1                   # 129: features + ones column

_BUILT = {}


def _build_nc(tpc):
    """Build the per-core Bass graph for `tpc` tiles of 128 rows."""
    from concourse import bacc, mybir, tile

    assert tpc % CG == 0
    ncg = tpc // CG          # chunk groups (8 for tpc=256)

    nc = bacc.Bacc("TRN2", target_bir_lowering=False, debug=False,
                   num_devices=NCORES)
    feat_ext = nc.declare_dram_parameter(
        "featp", [TILE, tpc, DP], mybir.dt.float16, isOutput=False)
    att_ext = nc.declare_dram_parameter(
        "att_t", [TILE, tpc * H], mybir.dt.float16, isOutput=False)
    part_ext = nc.declare_dram_parameter(
        "part", [TILE, ncg * DP], mybir.dt.float32, isOutput=True)

    with tile.TileContext(nc) as tc:
        with (
            tc.tile_pool(name="const", bufs=1) as cpool,
            tc.tile_pool(name="feat", bufs=3) as fpool,
            tc.tile_pool(name="psum", bufs=4, space="PSUM") as ppool,
        ):
            att_sb = cpool.tile([TILE, tpc * H], mybir.dt.float16)
            # first chunk-group's att slice first, so matmuls start early
            nc.scalar.dma_start(out=att_sb[:, :CG * H],
                                in_=att_ext.ap()[:, :CG * H])
            if tpc > CG:
                nc.scalar.dma_start(out=att_sb[:, CG * H:],
                                    in_=att_ext.ap()[:, CG * H:])
            out_sb = cpool.tile([TILE, ncg * DP], mybir.dt.float32)

            for cg in range(ncg):
                g0 = cg * CG
                feat_g = fpool.tile([TILE, CG, DP], mybir.dt.float16,
                                    tag="feat_g")
                if cg == 0:
                    # split first load so matmuls start sooner
                    for s in range(4):
                        nc.sync.dma_start(
                            out=feat_g[:, s * 8:(s + 1) * 8, :],
                            in_=feat_ext.ap()[:, s * 8:(s + 1) * 8, :],
                        )
                else:
                    nc.sync.dma_start(
                        out=feat_g[:, :, :],
                        in_=feat_ext.ap()[:, g0:g0 + CG, :],
                    )
                ps = ppool.tile([TILE, DP], mybir.dt.float32, tag="ps")
                # init the partitions between strips that no matmul writes,
                # so the full-partition copy below reads defined data
                nc.vector.memset(ps[:], 0.0)
                # each strip's accumulation group must fully close before
                # the next opens: start=True clears has_written for the
                # whole bank (values of closed groups are unaffected)
                for j in range(NSTRIP):
                    for q in range(Q):
                        tl = j * Q + q
                        t = g0 + tl
                        nc.tensor.matmul(
                            ps[32 * j:32 * j + H, :],
                            att_sb[:, t * H:(t + 1) * H],
                            feat_g[:, tl, :],
                            start=(q == 0),
                            stop=(q == Q - 1),
                            tile_position=(0, 32 * j),
                        )
                nc.vector.tensor_copy(out_sb[:, cg * DP:(cg + 1) * DP],
                                      ps[:])
                # stream each chunk-group's partials out as produced, so
                # only the last small DMA sits on the critical tail
                nc.scalar.dma_start(
                    out=part_ext.ap()[:, cg * DP:(cg + 1) * DP],
                    in_=out_sb[:, cg * DP:(cg + 1) * DP])
    nc.compile()
    return nc


def _get_nc(tpc):
    if tpc not in _BUILT:
        _BUILT[tpc] = _build_nc(tpc)
    return _BUILT[tpc]


def _prep_inputs(features, att, tpc):
    """Host-side pre-tiling into fp16 device layouts."""
    featp = np.empty((NCORES, TILE, tpc, DP), dtype=np.float16)
    featp[..., :D] = features.reshape(NCORES, tpc, TILE, D).transpose(
        0, 2, 1, 3)
    featp[..., D] = np.float16(1.0)
    att_t = np.ascontiguousarray(
        att.reshape(NCORES, tpc, TILE, H).transpose(0, 2, 1, 3)
    ).astype(np.float16).reshape(NCORES, TILE, tpc * H)
    return featp, att_t


def _extract_chunks(part, ncg):
    """part [128, ncg*DP] -> chunk partials [ncg*NSTRIP, H, DP] in row
    order (chunk index = cg*NSTRIP + j covers rows [idx*CHROWS ...))."""
    a = part.reshape(NSTRIP, 32, ncg, DP)   # [strip, sub-part, cg, col]
    return a[:, :H].transpose(2, 0, 1, 3).reshape(ncg * NSTRIP, H, DP)


def kernel(features, att, segment_ids, num_segments):
    from concourse.bass_utils import run_bass_kernel_spmd

    features = np.ascontiguousarray(np.asarray(features, dtype=np.float32))
    att = np.ascontiguousarray(np.asarray(att, dtype=np.float32))
    seg = np.asarray(segment_ids).astype(np.int64).ravel()
    B = int(num_segments)
    total = features.shape[0]
    assert total == TOTAL and features.shape[1] == D and att.shape[1] == H

    tpc = R // TILE  # 256
    ncg = tpc // CG
    nc = _get_nc(tpc)

    featp, att_t = _prep_inputs(features, att, tpc)
    in_maps = [{"featp": featp[c], "att_t": att_t[c]} for c in range(NCORES)]
    res = run_bass_kernel_spmd(nc, in_maps, core_ids=list(range(NCORES)),
                               trace=False)

    # [n_chunks_global, H, DP] chunk partials (CHROWS rows each)
    parts = np.concatenate([
        _extract_chunks(res.results[c]["part"], ncg) for c in range(NCORES)
    ])

    fws = np.zeros((B, H, D), dtype=np.float64)
    ws = np.zeros((B, H), dtype=np.float64)

    lo = seg[::CHROWS]
    hi = seg[CHROWS - 1::CHROWS]
    uniform = lo == hi

    # uniform chunks: bulk per-bag accumulation of device partials
    for b in range(B):
        m = uniform & (lo == b)
        if m.any():
            s = parts[m].sum(axis=0, dtype=np.float64)
            fws[b] += s[:, :D]
            ws[b] += s[:, D]

    # straddling chunks: recompute on host from raw rows
    for k in np.nonzero(~uniform)[0]:
        r0, r1 = k * CHROWS, (k + 1) * CHROWS
        sub_seg = seg[r0:r1]
        sub_att = att[r0:r1]
        sub_feat = features[r0:r1]
        for b in range(int(sub_seg[0]), int(sub_seg[-1]) + 1):
            rows = sub_seg == b
            if rows.any():
                a = sub_att[rows]
                fws[b] += a.T.astype(np.float64) @ sub_feat[rows]
                ws[b] += a.sum(axis=0, dtype=np.float64)

    fws = fws.astype(np.float32)
    ws = ws.astype(np.float32)
    denom = ws[:, :, None]
    safe = np.where(denom == 0, np.float32(1.0), denom)
    avg = np.where(denom == 0, np.float32(0.0), fws / safe)
    avg = np.where(np.isnan(avg), np.float32(1e-5), avg).astype(np.float32)
    return avg, ws


# revision 10
# speedup vs baseline: 1.0929x; 1.0929x over previous
"""Attention-based aggregation (ragged segment-sum) on 8 trn2 NeuronCores.

Strategy: shard the 262144-row instance dim evenly across 8 cores (32768
rows/core). Host pre-tiles features+att into fp16 SBUF-layout arrays
([partition, tile, col] with a ones column appended to features). Each core
streams 256 row-tiles through TensorE: matmul att_tile^T [128,8] @
[feat_tile | 1] [128,129] -> [8,129] fp32 in PSUM (weighted feature sums +
attention sums in one shot). PE column-tiling runs 4 such matmuls
concurrently in the 4 32-column strips of the array, each strip
accumulating Q=8 consecutive tiles -> one PSUM bank holds 4 chunk partials
of 1024 rows at partitions {0-7,32-39,64-71,96-103}. One DVE copy per bank
-> SBUF laid out across all 128 partitions -> single fast DMA out. Host
maps chunk partials onto bags (segment_ids are sorted, so only chunks
straddling a bag boundary are recomputed on host) and does the tiny
[B,8,128] division / NaN fixups.
"""

import numpy as np

TOTAL = 262144
D = 128
H = 8
NCORES = 8
R = TOTAL // NCORES          # rows per core = 32768
TILE = 128                   # rows per matmul tile (contraction dim)
Q = 8                        # tiles accumulated per PSUM chunk
CHROWS = TILE * Q            # 1024 rows per chunk
NSTRIP = 4                   # PE column strips used concurrently
CG = NSTRIP * Q              # tiles per chunk-group (= per PSUM bank) = 32
DP = D + 1                   # 129: features + ones column

_BUILT = {}


def _build_nc(tpc):
    """Build the per-core Bass graph for `tpc` tiles of 128 rows."""
    from concourse import bacc, mybir, tile

    assert tpc % CG == 0
    ncg = tpc // CG          # chunk groups (8 for tpc=256)

    nc = bacc.Bacc("TRN2", target_bir_lowering=False, debug=False,
                   num_devices=NCORES)
    feat_ext = nc.declare_dram_parameter(
        "featp", [TILE, tpc, DP], mybir.dt.float16, isOutput=False)
    att_ext = nc.declare_dram_parameter(
        "att_t", [TILE, tpc * H], mybir.dt.float16, isOutput=False)
    part_ext = nc.declare_dram_parameter(
        "part", [TILE, ncg * DP], mybir.dt.float32, isOutput=True)

    with tile.TileContext(nc) as tc:
        with (
            tc.tile_pool(name="const", bufs=1) as cpool,
            tc.tile_pool(name="feat", bufs=3) as fpool,
            tc.tile_pool(name="psum", bufs=4, space="PSUM") as ppool,
        ):
            att_sb = cpool.tile([TILE, tpc * H], mybir.dt.float16)
            nc.scalar.dma_start(out=att_sb[:], in_=att_ext.ap())
            out_sb = cpool.tile([TILE, ncg * DP], mybir.dt.float32)

            for cg in range(ncg):
                g0 = cg * CG
                feat_g = fpool.tile([TILE, CG, DP], mybir.dt.float16,
                                    tag="feat_g")
                if cg == 0:
                    # split first load so matmuls start sooner
                    for s in range(4):
                        nc.sync.dma_start(
                            out=feat_g[:, s * 8:(s + 1) * 8, :],
                            in_=feat_ext.ap()[:, s * 8:(s + 1) * 8, :],
                        )
                else:
                    nc.sync.dma_start(
                        out=feat_g[:, :, :],
                        in_=feat_ext.ap()[:, g0:g0 + CG, :],
                    )
                ps = ppool.tile([TILE, DP], mybir.dt.float32, tag="ps")
                # init the partitions between strips that no matmul writes,
                # so the full-partition copy below reads defined data
                nc.vector.memset(ps[:], 0.0)
                # each strip's accumulation group must fully close before
                # the next opens: start=True clears has_written for the
                # whole bank (values of closed groups are unaffected)
                for j in range(NSTRIP):
                    for q in range(Q):
                        tl = j * Q + q
                        t = g0 + tl
                        nc.tensor.matmul(
                            ps[32 * j:32 * j + H, :],
                            att_sb[:, t * H:(t + 1) * H],
                            feat_g[:, tl, :],
                            start=(q == 0),
                            stop=(q == Q - 1),
                            tile_position=(0, 32 * j),
                        )
                nc.vector.tensor_copy(out_sb[:, cg * DP:(cg + 1) * DP],
                                      ps[:])
            nc.scalar.dma_start(out=part_ext.ap(), in_=out_sb[:])
    nc.compile()
    return nc


def _get_nc(tpc):
    if tpc not in _BUILT:
        _BUILT[tpc] = _build_nc(tpc)
    return _BUILT[tpc]


def _prep_inputs(features, att, tpc):
    """Host-side pre-tiling into fp16 device layouts."""
    featp = np.empty((NCORES, TILE, tpc, DP), dtype=np.float16)
    featp[..., :D] = features.reshape(NCORES, tpc, TILE, D).transpose(
        0, 2, 1, 3)
    featp[..., D] = np.float16(1.0)
    att_t = np.ascontiguousarray(
        att.reshape(NCORES, tpc, TILE, H).transpose(0, 2, 1, 3)
    ).astype(np.float16).reshape(NCORES, TILE, tpc * H)
    return featp, att_t


def _extract_chunks(part, ncg):
    """part [128, ncg*DP] -> chunk partials [ncg*NSTRIP, H, DP] in row
    order (chunk index = cg*NSTRIP + j covers rows [idx*CHROWS ...))."""
    a = part.reshape(NSTRIP, 32, ncg, DP)   # [strip, sub-part, cg, col]
    return a[:, :H].transpose(2, 0, 1, 3).reshape(ncg * NSTRIP, H, DP)


def kernel(features, att, segment_ids, num_segments):
    from concourse.bass_utils import run_bass_kernel_spmd

    features = np.ascontiguousarray(np.asarray(features, dtype=np.float32))
    att = np.ascontiguousarray(np.asarray(att, dtype=np.float32))
    seg = np.asarray(segment_ids).astype(np.int64).ravel()
    B = int(num_segments)
    total = features.shape[0]
    assert total == TOTAL and features.shape[1] == D and att.shape[1] == H

    tpc = R // TILE  # 256
    ncg = tpc // CG
    nc = _get_nc(tpc)

    featp, att_t = _prep_inputs(features, att, tpc)
    in_maps = [{"featp": featp[c], "att_t": att_t[c]} for c in range(NCORES)]
    res = run_bass_kernel_spmd(nc, in_maps, core_ids=list(range(NCORES)),
                               trace=False)

    # [n_chunks_global, H, DP] chunk partials (CHROWS rows each)
    parts = np.concatenate([
        _extract_chunks(res.results[c]["part"], ncg) for c in range(NCORES)
    ])

    fws = np.zeros((B, H, D), dtype=np.float64)
    ws = np.zeros((B, H), dtype=np.float64)

    lo = seg[::CHROWS]
    hi = seg[CHROWS - 1::CHROWS]
    uniform = lo == hi

    # uniform chunks: bulk per-bag accumulation of device partials
    for b in range(B):
        m = uniform & (lo == b)
        if m.any():
            s = parts[m].sum(axis=0, dtype=np.float64)
            fws[b] += s[:, :D]
            ws[b] += s[:, D]

    # straddling chunks: recompute on host from raw rows
    for k in np.nonzero(~uniform)[0]:
        r0, r1 = k * CHROWS, (k + 1) * CHROWS
        sub_seg = seg[r0:r1]
        sub_att = att[r0:r1]
        sub_feat = features[r0:r1]
        for b in range(int(sub_seg[0]), int(sub_seg[-1]) + 1):
            rows = sub_seg == b
            if rows.any():
                a = sub_att[rows]
                fws[b] += a.T.astype(np.float64) @ sub_feat[rows]
                ws[b] += a.sum(axis=0, dtype=np.float64)

    fws = fws.astype(np.float32)
    ws = ws.astype(np.float32)
    denom = ws[:, :, None]
    safe = np.where(denom == 0, np.float32(1.0), denom)
    avg = np.where(denom == 0, np.float32(0.0), fws / safe)
    avg = np.where(np.isnan(avg), np.float32(1e-5), avg).astype(np.float32)
    return avg, ws
